# revision 3
# baseline (speedup 1.0000x reference)
"""Chamfer loss kernel for Trainium2 (Bass/Tile), 8 NeuronCores.

Math: for each batch b, D_b[n, m] = ||pred[b,n] - label[b,m]||.
result = mean_n(min_m D) + mean_m(min_n D).

Strategy
--------
Sharding: 8 cores = 4 batches x 2 halves of the pred axis. Core c
(b = c//2, h = c%2) owns queries q = pred[b, h*4096:(h+1)*4096] (NQ=4096)
and all refs r = label[b] (NR=8192). Each core makes ONE pass over its
4096 x 8192 block of the (negated) squared-distance matrix and produces
BOTH reductions from that single pass:
  - per-ref  max of -d^2 over its 4096 queries -> partial min_n; the two
    halves of a batch are combined on the host (tiny elementwise max).
  - per-query max of -d^2 over all 8192 refs -> complete min_m for its
    4096 pred points (finished on device: relu, sqrt, partial sums).

PE: -d^2 = 2 q.r - ||q||^2 - ||r||^2 as a K=16 bf16 matmul using the
split-bf16 trick (q ~ qh+ql, r ~ rh+rl, norms split hi/lo as well), so
products are exact bf16xbf16 accumulated in fp32 -> ~fp32 accuracy at
1 cycle/row. Stationary = 128 refs per row tile, moving = 512 queries
per matmul, 4 independent 16-row problems packed in the 128-partition
array via tile_position quadrants.

Wall-clock focus: with profiling unavailable, the graded time is the
wall clock of a full kernel() call through the axon tunnel, which is
dominated by host<->device transfer bandwidth and per-RPC latency, not
device compute. So:
  - inputs ship as ONE compact [8, 12288] bf16 tensor per core (~192 KB:
    hi/lo coordinate rows + hi/lo norm rows for q and r); the replicated
    128-partition matmul strips are assembled on-device by DMA re-reads
    of the same DRAM rows (the old layout shipped the pre-replicated
    strips, 12x more bytes).
  - the static 128x128 identity (PE transpose operand) is staged onto
    each device once at setup, outside the per-call path.
  - both outputs pack into one [128, 65] f32 tensor (cols 0-63 per-ref
    max partials, col 64 per-query sqrt-sum partials) -> one fetch RPC.
  - dispatch is 8 independent single-device jit calls issued from a
    thread pool (put -> exec -> fetch pipelined per device, overlapped
    across devices), instead of one shard_map launch whose 8 per-device
    RPCs serialize.

kernel(pred, label) takes the full inputs, preps the compact augmented
layouts on host (cheap O(N*D) numpy), runs the 8 cores, and combines the
small per-core outputs.
"""

import os
import sys
from concurrent.futures import ThreadPoolExecutor

import numpy as np

for _p in ("/opt/trn_rl_repo", "/root/.axon_site/_ro/trn_rl_repo"):
    if os.path.isdir(_p) and _p not in sys.path:
        sys.path.append(_p)

import ml_dtypes

import concourse.bacc as bacc
import concourse.mybir as mybir
from concourse import tile

F32 = mybir.dt.float32
BF16 = mybir.dt.bfloat16
NPBF16 = ml_dtypes.bfloat16
OP_MAX = mybir.AluOpType.max
AX_X = mybir.AxisListType.X
SQRT = mybir.ActivationFunctionType.Sqrt
COPY = mybir.ActivationFunctionType.Copy

B = 4
N = 8192
NCORES = 8
NEG16 = -60000.0

NQ = N // 2      # queries per core (pred half)
NR = N           # refs per core (all labels of the batch)
MMN = 512        # moving free dim per matmul (one PSUM bank)
K = 16           # split-bf16 augmented contraction dim
RT = NR // 128   # ref row-tiles (64)


def build_program(nq=NQ, nr=NR, mmn=MMN, dve_copy_every=5, scp_bufs=4,
                  red_mode=1, gmm=4):
    """Emit + compile the per-core program.

    Input: aug [8, nq+nr] bf16. Columns 0:nq are the query block, rows
    {qh.T(0-2), ql.T(3-5), q2h(6), q2l(7)}; columns nq:nq+nr the ref
    block, rows {2rh.T(0-2), 2rl.T(3-5), -r2h(6), -r2l(7)}. The
    replicated 4-quadrant strips the PE needs are built here by DMA.
    Output: out [128, rt+1] f32 = [RMS | qsum].
    """
    nchunk = gmm * mmn             # columns per consume group
    ngroup = nq // nchunk          # consume groups per ref row-tile
    rt = nr // 128                 # ref row-tiles
    psum_bufs = 8 // gmm           # PSUM slots (gmm banks each)
    assert nq % nchunk == 0 and nr % 128 == 0 and nq % 128 == 0

    S16 = BF16
    nc = bacc.Bacc("TRN2", target_bir_lowering=False, debug=False)
    aug_d = nc.dram_tensor("aug", [8, nq + nr], BF16, kind="ExternalInput")
    id_d = nc.dram_tensor("ident", [128, 128], F32, kind="ExternalInput")
    out_d = nc.dram_tensor("out", [128, rt + 1], F32, kind="ExternalOutput")

    with tile.TileContext(nc) as tc:
        with (
            tc.tile_pool(name="const", bufs=1) as const,
            tc.tile_pool(name="rmp", bufs=2) as rmp,
            tc.tile_pool(name="scp", bufs=scp_bufs) as scp,
            tc.tile_pool(name="tail", bufs=1) as tail,
        ):
            QS = const.tile([128, nq // 4], BF16)
            RS = const.tile([128, nr], BF16)
            IDENT = const.tile([128, 128], F32)
            nc.sync.dma_start(IDENT[:], id_d.ap())

            # Strip assembly. qs[32s+k, j2*mmn+m] = qaug[k, (4j2+s)*mmn+m]
            # with qaug rows [qh(3), ql(3), qh(3), ql(3), 1, 1, q2h, q2l];
            # rs[32s+k, :] = raug[k, :] with raug rows
            # [2rh(3), 2rh(3), 2rl(3), 2rl(3), -r2h, -r2l, -1, -1].
            # engines can't address partition slices off the 32-partition
            # grid, so pre-fill the constant rows (1.0 in qs, -1.0 in rs)
            # by memsetting the whole tiles, then DMA the data rows over.
            nc.vector.memset(QS[:], 1.0)
            nc.vector.memset(RS[:], -1.0)
            qb = aug_d.ap()[0:8, 0:nq].rearrange(
                "r (g four m) -> r g four m", four=4, m=mmn)
            rb = aug_d.ap()[0:8, nq:nq + nr]
            for s in range(4):
                qdst = QS[32 * s:32 * s + 16, :].rearrange(
                    "p (g m) -> p g m", m=mmn)
                nc.sync.dma_start(qdst[0:6], qb[0:6, :, s, :])
                nc.sync.dma_start(qdst[6:12], qb[0:6, :, s, :])
                nc.sync.dma_start(qdst[14:16], qb[6:8, :, s, :])
                nc.sync.dma_start(RS[32 * s + 0:32 * s + 3, :], rb[0:3, :])
                nc.sync.dma_start(RS[32 * s + 3:32 * s + 6, :], rb[0:3, :])
                nc.sync.dma_start(RS[32 * s + 6:32 * s + 9, :], rb[3:6, :])
                nc.sync.dma_start(RS[32 * s + 9:32 * s + 12, :], rb[3:6, :])
                nc.sync.dma_start(RS[32 * s + 12:32 * s + 14, :], rb[6:8, :])

            CM = const.tile([128, nq], S16)
            RMS = const.tile([128, rt], F32)

            with tc.tile_pool(name="psum", bufs=psum_bufs, space="PSUM") as psum:
                nc.vector.memset(CM[:], NEG16)
                for r in range(rt):
                    # one contiguous copy target for the whole row tile
                    sc = scp.tile([128, nq], S16, tag="sc")
                    dve_rt = (ngroup > 1 and dve_copy_every
                              and r % dve_copy_every == 0)
                    rg0 = None
                    for j2 in range(ngroup):
                        ps = psum.tile([128, nchunk], F32)
                        for i in range(gmm):
                            nc.tensor.matmul(
                                ps[:, i * mmn:(i + 1) * mmn],
                                RS[32 * i:32 * i + K, r * 128:(r + 1) * 128],
                                QS[32 * i:32 * i + K, j2 * mmn:(j2 + 1) * mmn],
                                start=True,
                                stop=True,
                                tile_position=(32 * i, 0),
                            )
                        sc_sl = sc[:, j2 * nchunk:(j2 + 1) * nchunk]
                        if dve_rt and j2 == 0:
                            # DVE reads this PSUM group: fused copy+reduce
                            rg0 = rmp.tile([128, 1], F32, tag="rg0")
                            nc.vector.tensor_scalar(
                                sc_sl, ps[:], -3.0e38, None, OP_MAX, OP_MAX,
                                accum_out=rg0[:])
                        else:
                            nc.scalar.activation(sc_sl, ps[:], COPY)
                    # per-ref reduce over the whole row tile
                    if dve_rt:
                        rg1 = rmp.tile([128, 1], F32, tag="rg1")
                        nc.vector.reduce_max(rg1[:], sc[:, nchunk:], axis=AX_X)
                        nc.vector.tensor_max(RMS[:, r:r + 1], rg0[:], rg1[:])
                    else:
                        nc.vector.reduce_max(RMS[:, r:r + 1], sc[:], axis=AX_X)
                    # per-query fold into CM (full row-tile width)
                    nc.vector.tensor_max(CM[:], CM[:], sc[:])

                nc.sync.dma_start(out_d.ap()[:, 0:rt], RMS[:])

                # per-query direction: max over the 128 partitions of CM.
                # Clamp+convert CM16 -> fp32, PE-transpose each 128x128
                # block into PSUM, reduce_max over the candidate axis,
                # then sqrt of the negated minima and sum.
                CM32 = tail.tile([128, nq], F32)
                nc.vector.tensor_scalar_min(CM32[:], CM[:], 0.0)
                nblk = nq // 128
                nbp = nchunk // 128    # transpose blocks per pass
                q2 = tail.tile([128, nblk], F32)
                for h2 in range(nq // nchunk):
                    pst = psum.tile([128, nchunk], F32, tag="ps")
                    for bb in range(nbp):
                        blk = h2 * nbp + bb
                        nc.tensor.transpose(
                            pst[:, bb * 128:(bb + 1) * 128],
                            CM32[:, blk * 128:(blk + 1) * 128],
                            IDENT[:],
                        )
                    nc.vector.tensor_reduce(
                        q2[:, h2 * nbp:(h2 + 1) * nbp],
                        pst[:].rearrange("p (b c) -> p b c", c=128),
                        axis=AX_X, op=OP_MAX,
                    )
                # q2 holds v = max(-d^2) clamped <= 0; sqrt(-v) = distance.
                sq = tail.tile([128, nblk], F32)
                nc.scalar.activation(sq[:], q2[:], SQRT, bias=0.0, scale=-1.0)
                qsum = tail.tile([128, 1], F32)
                nc.vector.reduce_sum(qsum[:], sq[:], axis=AX_X)
                nc.sync.dma_start(out_d.ap()[:, rt:rt + 1], qsum[:])

    nc.compile()
    return nc


def _split2(x):
    """fp32 -> (hi, lo) fp32 arrays exactly representable in bf16."""
    hi = x.astype(NPBF16).astype(np.float32)
    lo = (x - hi).astype(NPBF16).astype(np.float32)
    return hi, lo


def make_augs(pred, label):
    """Compact per-core augmented layouts: [8, NQ+NR] bf16 per core."""
    pred = np.asarray(pred, np.float32)
    label = np.asarray(label, np.float32)
    augs = []
    for b in range(B):
        r = np.ascontiguousarray(label[b])
        rh, rl = _split2(r)
        rtt = rh + rl
        r2h, r2l = _split2((rtt * rtt).sum(1, dtype=np.float32))
        rblock = np.empty((8, NR), np.float32)
        rblock[0:3] = (2.0 * rh).T
        rblock[3:6] = (2.0 * rl).T
        rblock[6] = -r2h
        rblock[7] = -r2l
        rblock16 = rblock.astype(NPBF16)
        for h in range(2):
            q = np.ascontiguousarray(pred[b, h * NQ:(h + 1) * NQ])
            qh, ql = _split2(q)
            qt = qh + ql
            q2h, q2l = _split2((qt * qt).sum(1, dtype=np.float32))
            qblock = np.empty((8, NQ), np.float32)
            qblock[0:3] = qh.T
            qblock[3:6] = ql.T
            qblock[6] = q2h
            qblock[7] = q2l
            aug = np.empty((8, NQ + NR), NPBF16)
            aug[:, :NQ] = qblock.astype(NPBF16)
            aug[:, NQ:] = rblock16
            augs.append(aug)
    return augs


def postprocess(outs):
    """outs: list of 8 [128, RT+1] f32 arrays ([RMS | qsum])."""
    sq_sum = sum(float(o[:, RT].sum(dtype=np.float64)) for o in outs)
    ref_sum = 0.0
    for b in range(B):
        m = np.maximum(outs[2 * b][:, :RT], outs[2 * b + 1][:, :RT])
        ref_sum += float(np.sqrt(np.maximum(-m, 0.0)).sum(dtype=np.float64))
    return np.float32((sq_sum + ref_sum) / (B * N))


_CTX = None


def _setup():
    global _CTX
    if _CTX is not None:
        return _CTX
    import jax
    from concourse.bass2jax import (_bass_exec_p, install_neuronx_cc_hook,
                                    partition_id_tensor)

    nc = build_program()
    install_neuronx_cc_hook()

    partition_name = (nc.partition_id_tensor.name
                      if nc.partition_id_tensor else None)
    in_names, out_names, out_avals = [], [], []
    for alloc in nc.m.functions[0].allocations:
        if not isinstance(alloc, mybir.MemoryLocationSet):
            continue
        name = alloc.memorylocations[0].name
        if alloc.kind == "ExternalInput":
            if name != partition_name:
                in_names.append(name)
        elif alloc.kind == "ExternalOutput":
            shape = tuple(alloc.tensor_shape)
            dtype = mybir.dt.np(alloc.dtype)
            out_names.append(name)
            out_avals.append(jax.core.ShapedArray(shape, dtype))
    assert in_names == ["aug", "ident"] and out_names == ["out"], (
        in_names, out_names)
    all_in_names = in_names + out_names + (
        [partition_name] if partition_name else [])
    n_params = len(in_names)
    donate = tuple(range(n_params, n_params + len(out_names)))

    def _body(*args):
        operands = list(args)
        if partition_name is not None:
            operands.append(partition_id_tensor())
        outs = _bass_exec_p.bind(
            *operands, out_avals=tuple(out_avals),
            in_names=tuple(all_in_names), out_names=tuple(out_names),
            lowering_input_output_aliases=(), sim_require_finite=True,
            sim_require_nnan=True, nc=nc)
        return tuple(outs)

    jfn = jax.jit(_body, donate_argnums=donate, keep_unused=True)
    devices = jax.devices()[:NCORES]
    ident = np.eye(128, dtype=np.float32)
    ident_devs = [jax.device_put(ident, d) for d in devices]
    # warm/compile each device executable once (NEFF compile is cached)
    dummy = np.zeros((8, NQ + NR), NPBF16)
    for c in range(NCORES):
        ad = jax.device_put(dummy, devices[c])
        (o,) = jfn(ad, ident_devs[c], np.zeros((128, RT + 1), np.float32))
        np.asarray(o)
    pool = ThreadPoolExecutor(NCORES)
    _CTX = {
        "jax": jax, "jfn": jfn, "devices": devices,
        "ident_devs": ident_devs, "pool": pool,
    }
    return _CTX


def kernel(pred, label):
    ctx = _setup()
    jax, jfn = ctx["jax"], ctx["jfn"]
    devices, ident_devs = ctx["devices"], ctx["ident_devs"]
    augs = make_augs(pred, label)

    def run_core(c):
        ad = jax.device_put(augs[c], devices[c])
        (o,) = jfn(ad, ident_devs[c], np.zeros((128, RT + 1), np.float32))
        return np.asarray(o)

    outs = list(ctx["pool"].map(run_core, range(NCORES)))
    return postprocess(outs)


# revision 4
# speedup vs baseline: 1.1131x; 1.1131x over previous
"""Chamfer loss kernel for Trainium2 (Bass/Tile), 8 NeuronCores.

Math: for each batch b, D_b[n, m] = ||pred[b,n] - label[b,m]||.
result = mean_n(min_m D) + mean_m(min_n D).

Strategy
--------
Sharding: 8 cores = 4 batches x 2 halves of the pred axis. Core c
(b = c//2, h = c%2) owns queries q = pred[b, h*4096:(h+1)*4096] (NQ=4096)
and all refs r = label[b] (NR=8192). Each core makes ONE pass over its
4096 x 8192 block of the (negated) squared-distance matrix and produces
BOTH reductions from that single pass:
  - per-ref  max of -d^2 over its 4096 queries -> partial min_n; the two
    halves of a batch are combined on the host (tiny elementwise max).
  - per-query max of -d^2 over all 8192 refs -> complete min_m for its
    4096 pred points (finished on device: relu, sqrt, partial sums).

PE: -d^2 = 2 q.r - ||q||^2 - ||r||^2 as a K=7 fp16 matmul: coordinates
rounded to fp16 (points move by ~3e-4, Chamfer changes by ~2e-5 rel --
far inside the 2e-2 gate), norms computed FROM the rounded coordinates
and carried as an fp16 hi/lo pair so the Gram identity cancels exactly:
the PSUM value is |q~ - r~|^2 to fp32 accuracy. Products are exact
fp16xfp16 in fp32. Stationary = 128 refs per row tile, moving = 512
queries per matmul, 4 independent 7-row problems packed in the
128-partition array via tile_position quadrants.

Wall-clock focus: with profiling unavailable, the graded time is the
wall clock of a full kernel() call through the axon tunnel, which is
dominated by per-RPC latency (~40-80ms response quantum) and transfer
bandwidth, not device compute (<1ms). So:
  - inputs ship as ONE compact [5, 12288] fp16 tensor per core (120 KB:
    3 coordinate rows + 2 norm rows for q and r); the replicated
    128-partition matmul strips are assembled on-device by DMA re-reads
    of the same DRAM rows. (The original layout shipped pre-replicated
    bf16-split strips: 2.4 MB per core, 20x more.)
  - the static 128x128 identity (PE transpose operand) is staged onto
    each device once at setup, outside the per-call path.
  - both outputs pack into one [128, 65] f32 tensor (cols 0-63 per-ref
    max partials, col 64 per-query sqrt-sum partials) -> one fetch RPC.
  - dispatch is 8 independent single-device jit calls issued from a
    thread pool (put -> exec -> fetch pipelined per device, overlapped
    across devices); the donated zero output buffers are prefetched
    and each core is submitted as soon as its host prep finishes.

kernel(pred, label) takes the full inputs, preps the compact augmented
layouts on host (cheap O(N*D) numpy), runs the 8 cores, and combines the
small per-core outputs.
"""

import os
import sys
from concurrent.futures import ThreadPoolExecutor

import numpy as np

for _p in ("/opt/trn_rl_repo", "/root/.axon_site/_ro/trn_rl_repo"):
    if os.path.isdir(_p) and _p not in sys.path:
        sys.path.append(_p)

import concourse.bacc as bacc
import concourse.mybir as mybir
from concourse import tile

F32 = mybir.dt.float32
F16 = mybir.dt.float16
BF16 = mybir.dt.bfloat16
OP_MAX = mybir.AluOpType.max
AX_X = mybir.AxisListType.X
SQRT = mybir.ActivationFunctionType.Sqrt
COPY = mybir.ActivationFunctionType.Copy

B = 4
N = 8192
NCORES = 8
NEG16 = -60000.0

NQ = N // 2      # queries per core (pred half)
NR = N           # refs per core (all labels of the batch)
MMN = 512        # moving free dim per matmul (one PSUM bank)
K = 7            # fp16 augmented contraction dim
RT = NR // 128   # ref row-tiles (64)


def build_program(nq=NQ, nr=NR, mmn=MMN, dve_copy_every=5, scp_bufs=4,
                  gmm=4):
    """Emit + compile the per-core program.

    Input: aug [5, nq+nr] fp16. Columns 0:nq are the query block, rows
    {q.T(0-2), q2h(3), q2l(4)}; columns nq:nq+nr the ref block, rows
    {2r.T(0-2), -r2h(3), -r2l(4)}. The replicated 4-quadrant K=7 strips
    the PE needs (q side [q(3), 1, 1, q2h, q2l], r side
    [2r(3), -r2h, -r2l, -1, -1]) are built here by DMA + memset.
    Output: out [128, rt+1] f32 = [RMS | qsum].
    """
    nchunk = gmm * mmn             # columns per consume group
    ngroup = nq // nchunk          # consume groups per ref row-tile
    rt = nr // 128                 # ref row-tiles
    psum_bufs = 8 // gmm           # PSUM slots (gmm banks each)
    assert nq % nchunk == 0 and nr % 128 == 0 and nq % 128 == 0

    S16 = BF16
    nc = bacc.Bacc("TRN2", target_bir_lowering=False, debug=False)
    aug_d = nc.dram_tensor("aug", [5, nq + nr], F16, kind="ExternalInput")
    id_d = nc.dram_tensor("ident", [128, 128], F32, kind="ExternalInput")
    out_d = nc.dram_tensor("out", [128, rt + 1], F32, kind="ExternalOutput")

    with tile.TileContext(nc) as tc:
        with (
            tc.tile_pool(name="const", bufs=1) as const,
            tc.tile_pool(name="rmp", bufs=2) as rmp,
            tc.tile_pool(name="scp", bufs=scp_bufs) as scp,
            tc.tile_pool(name="tail", bufs=1) as tail,
        ):
            QS = const.tile([128, nq // 4], F16)
            RS = const.tile([128, nr], F16)
            IDENT = const.tile([128, 128], F32)
            nc.sync.dma_start(IDENT[:], id_d.ap())

            # Strip assembly. qs[32s+k, j2*mmn+m] = qaug[k, (4j2+s)*mmn+m]
            # with qaug rows [q(3), 1, 1, q2h, q2l]; rs[32s+k, :] =
            # raug[k, :] with raug rows [2r(3), -r2h, -r2l, -1, -1].
            # Engines can't address partition slices off the 32-partition
            # grid, so pre-fill the constant rows by memsetting the whole
            # tiles, then DMA the data rows over (DMA has no partition
            # alignment constraint).
            nc.vector.memset(QS[:], 1.0)
            nc.vector.memset(RS[:], -1.0)
            qb = aug_d.ap()[0:5, 0:nq].rearrange(
                "r (g four m) -> r g four m", four=4, m=mmn)
            rb = aug_d.ap()[0:5, nq:nq + nr]
            for s in range(4):
                qdst = QS[32 * s:32 * s + 8, :].rearrange(
                    "p (g m) -> p g m", m=mmn)
                nc.sync.dma_start(qdst[0:3], qb[0:3, :, s, :])
                nc.sync.dma_start(qdst[5:7], qb[3:5, :, s, :])
                nc.sync.dma_start(RS[32 * s + 0:32 * s + 3, :], rb[0:3, :])
                nc.sync.dma_start(RS[32 * s + 3:32 * s + 5, :], rb[3:5, :])

            CM = const.tile([128, nq], S16)
            RMS = const.tile([128, rt], F32)

            with tc.tile_pool(name="psum", bufs=psum_bufs, space="PSUM") as psum:
                nc.vector.memset(CM[:], NEG16)
                for r in range(rt):
                    # one contiguous copy target for the whole row tile
                    sc = scp.tile([128, nq], S16, tag="sc")
                    dve_rt = (ngroup > 1 and dve_copy_every
                              and r % dve_copy_every == 0)
                    rg0 = None
                    for j2 in range(ngroup):
                        ps = psum.tile([128, nchunk], F32)
                        for i in range(gmm):
                            nc.tensor.matmul(
                                ps[:, i * mmn:(i + 1) * mmn],
                                RS[32 * i:32 * i + K, r * 128:(r + 1) * 128],
                                QS[32 * i:32 * i + K, j2 * mmn:(j2 + 1) * mmn],
                                start=True,
                                stop=True,
                                tile_position=(32 * i, 0),
                            )
                        sc_sl = sc[:, j2 * nchunk:(j2 + 1) * nchunk]
                        if dve_rt and j2 == 0:
                            # DVE reads this PSUM group: fused copy+reduce
                            rg0 = rmp.tile([128, 1], F32, tag="rg0")
                            nc.vector.tensor_scalar(
                                sc_sl, ps[:], -3.0e38, None, OP_MAX, OP_MAX,
                                accum_out=rg0[:])
                        else:
                            nc.scalar.activation(sc_sl, ps[:], COPY)
                    # per-ref reduce over the whole row tile
                    if dve_rt:
                        rg1 = rmp.tile([128, 1], F32, tag="rg1")
                        nc.vector.reduce_max(rg1[:], sc[:, nchunk:], axis=AX_X)
                        nc.vector.tensor_max(RMS[:, r:r + 1], rg0[:], rg1[:])
                    else:
                        nc.vector.reduce_max(RMS[:, r:r + 1], sc[:], axis=AX_X)
                    # per-query fold into CM (full row-tile width)
                    nc.vector.tensor_max(CM[:], CM[:], sc[:])

                nc.sync.dma_start(out_d.ap()[:, 0:rt], RMS[:])

                # per-query direction: max over the 128 partitions of CM.
                # Clamp+convert CM16 -> fp32, PE-transpose each 128x128
                # block into PSUM, reduce_max over the candidate axis,
                # then sqrt of the negated minima and sum.
                CM32 = tail.tile([128, nq], F32)
                nc.vector.tensor_scalar_min(CM32[:], CM[:], 0.0)
                nblk = nq // 128
                nbp = nchunk // 128    # transpose blocks per pass
                q2 = tail.tile([128, nblk], F32)
                for h2 in range(nq // nchunk):
                    pst = psum.tile([128, nchunk], F32, tag="ps")
                    for bb in range(nbp):
                        blk = h2 * nbp + bb
                        nc.tensor.transpose(
                            pst[:, bb * 128:(bb + 1) * 128],
                            CM32[:, blk * 128:(blk + 1) * 128],
                            IDENT[:],
                        )
                    nc.vector.tensor_reduce(
                        q2[:, h2 * nbp:(h2 + 1) * nbp],
                        pst[:].rearrange("p (b c) -> p b c", c=128),
                        axis=AX_X, op=OP_MAX,
                    )
                # q2 holds v = max(-d^2) clamped <= 0; sqrt(-v) = distance.
                sq = tail.tile([128, nblk], F32)
                nc.scalar.activation(sq[:], q2[:], SQRT, bias=0.0, scale=-1.0)
                qsum = tail.tile([128, 1], F32)
                nc.vector.reduce_sum(qsum[:], sq[:], axis=AX_X)
                nc.sync.dma_start(out_d.ap()[:, rt:rt + 1], qsum[:])

    nc.compile()
    return nc


def _split2_16(x):
    """fp32 -> (hi, lo) fp32 arrays exactly representable in fp16."""
    hi = x.astype(np.float16).astype(np.float32)
    lo = (x - hi).astype(np.float16).astype(np.float32)
    return hi, lo


def iter_augs(pred, label):
    """Yield (core, aug [5, NQ+NR] fp16) in core order."""
    pred = np.asarray(pred, np.float32)
    label = np.asarray(label, np.float32)
    for b in range(B):
        r16 = np.ascontiguousarray(label[b]).astype(np.float16)
        r32 = r16.astype(np.float32)
        r2h, r2l = _split2_16((r32 * r32).sum(1, dtype=np.float32))
        rblock = np.empty((5, NR), np.float16)
        rblock[0:3] = 2.0 * r16.T
        rblock[3] = -r2h
        rblock[4] = -r2l
        for h in range(2):
            q16 = np.ascontiguousarray(
                pred[b, h * NQ:(h + 1) * NQ]).astype(np.float16)
            q32 = q16.astype(np.float32)
            q2h, q2l = _split2_16((q32 * q32).sum(1, dtype=np.float32))
            aug = np.empty((5, NQ + NR), np.float16)
            aug[0:3, :NQ] = q16.T
            aug[3, :NQ] = q2h
            aug[4, :NQ] = q2l
            aug[:, NQ:] = rblock
            yield 2 * b + h, aug


def postprocess(outs):
    """outs: list of 8 [128, RT+1] f32 arrays ([RMS | qsum])."""
    sq_sum = sum(float(o[:, RT].sum(dtype=np.float64)) for o in outs)
    ref_sum = 0.0
    for b in range(B):
        m = np.maximum(outs[2 * b][:, :RT], outs[2 * b + 1][:, :RT])
        ref_sum += float(np.sqrt(np.maximum(-m, 0.0)).sum(dtype=np.float64))
    return np.float32((sq_sum + ref_sum) / (B * N))


_CTX = None


def _setup():
    global _CTX
    if _CTX is not None:
        return _CTX
    import jax
    from concourse.bass2jax import (_bass_exec_p, install_neuronx_cc_hook,
                                    partition_id_tensor)

    nc = build_program()
    install_neuronx_cc_hook()

    partition_name = (nc.partition_id_tensor.name
                      if nc.partition_id_tensor else None)
    in_names, out_names, out_avals = [], [], []
    for alloc in nc.m.functions[0].allocations:
        if not isinstance(alloc, mybir.MemoryLocationSet):
            continue
        name = alloc.memorylocations[0].name
        if alloc.kind == "ExternalInput":
            if name != partition_name:
                in_names.append(name)
        elif alloc.kind == "ExternalOutput":
            shape = tuple(alloc.tensor_shape)
            dtype = mybir.dt.np(alloc.dtype)
            out_names.append(name)
            out_avals.append(jax.core.ShapedArray(shape, dtype))
    assert in_names == ["aug", "ident"] and out_names == ["out"], (
        in_names, out_names)
    all_in_names = in_names + out_names + (
        [partition_name] if partition_name else [])
    n_params = len(in_names)
    donate = tuple(range(n_params, n_params + len(out_names)))

    def _body(*args):
        operands = list(args)
        if partition_name is not None:
            operands.append(partition_id_tensor())
        outs = _bass_exec_p.bind(
            *operands, out_avals=tuple(out_avals),
            in_names=tuple(all_in_names), out_names=tuple(out_names),
            lowering_input_output_aliases=(), sim_require_finite=True,
            sim_require_nnan=True, nc=nc)
        return tuple(outs)

    jfn = jax.jit(_body, donate_argnums=donate, keep_unused=True)
    devices = jax.devices()[:NCORES]
    ident = np.eye(128, dtype=np.float32)
    ident_devs = [jax.device_put(ident, d) for d in devices]
    # warm/compile each device executable once (NEFF compile is cached)
    dummy = np.zeros((5, NQ + NR), np.float16)
    for c in range(NCORES):
        ad = jax.device_put(dummy, devices[c])
        (o,) = jfn(ad, ident_devs[c], np.zeros((128, RT + 1), np.float32))
        np.asarray(o)
    pool = ThreadPoolExecutor(NCORES)
    _CTX = {
        "jax": jax, "jfn": jfn, "devices": devices,
        "ident_devs": ident_devs, "pool": pool,
        "zeros": np.zeros((128, RT + 1), np.float32),
    }
    return _CTX


def kernel(pred, label):
    ctx = _setup()
    jax, jfn = ctx["jax"], ctx["jfn"]
    devices, ident_devs = ctx["devices"], ctx["ident_devs"]
    pool, zeros = ctx["pool"], ctx["zeros"]

    # prefetch the donated zero output buffers (input-independent) so
    # their transfer streams while host prep runs
    zfuts = [pool.submit(jax.device_put, zeros, devices[c])
             for c in range(NCORES)]

    def run_core(c, aug):
        ad = jax.device_put(aug, devices[c])
        (o,) = jfn(ad, ident_devs[c], zfuts[c].result())
        return np.asarray(o)

    # submit each core as soon as its host prep is done
    futs = [None] * NCORES
    for c, aug in iter_augs(pred, label):
        futs[c] = pool.submit(run_core, c, aug)
    outs = [f.result() for f in futs]
    return postprocess(outs)


# revision 8
# speedup vs baseline: 1.2026x; 1.0804x over previous
"""Chamfer loss kernel for Trainium2 (Bass/Tile), 8 NeuronCores.

Math: for each batch b, D_b[n, m] = ||pred[b,n] - label[b,m]||.
result = mean_n(min_m D) + mean_m(min_n D).

Strategy
--------
Sharding: 8 cores = 4 batches x 2 halves of the pred axis. Core c
(b = c//2, h = c%2) owns queries q = pred[b, h*4096:(h+1)*4096] (NQ=4096)
and all refs r = label[b] (NR=8192). Each core makes ONE pass over its
4096 x 8192 block of the (negated) squared-distance matrix and produces
BOTH reductions from that single pass:
  - per-ref  max of -d^2 over its 4096 queries -> partial min_n; the two
    halves of a batch are combined on the host (tiny elementwise max).
  - per-query max of -d^2 over all 8192 refs -> complete min_m for its
    4096 pred points (finished on device: relu, sqrt, partial sums).

PE: -d^2 = 2 q.r - ||q||^2 - ||r||^2 as a K=7 fp16 matmul: coordinates
rounded to fp16 (points move by ~3e-4, Chamfer changes by ~2e-5 rel --
far inside the 2e-2 gate), norms computed FROM the rounded coordinates
and carried as an fp16 hi/lo pair so the Gram identity cancels exactly:
the PSUM value is |q~ - r~|^2 to fp32 accuracy. Products are exact
fp16xfp16 in fp32. Stationary = 128 refs per row tile, moving = 512
queries per matmul, 4 independent 7-row problems packed in the
128-partition array via tile_position quadrants.

Wall-clock focus: with profiling unavailable, the graded time is the
wall clock of a full kernel() call through the axon tunnel, which is
dominated by per-RPC latency (~40-80ms response quantum) and transfer
bandwidth, not device compute (<1ms). So:
  - inputs ship as ONE compact [5, 12288] fp16 tensor per core (120 KB:
    3 coordinate rows + 2 norm rows for q and r); the replicated
    128-partition matmul strips are assembled on-device by DMA re-reads
    of the same DRAM rows. (The original layout shipped pre-replicated
    bf16-split strips: 2.4 MB per core, 20x more.)
  - the static 128x128 identity (PE transpose operand) is staged onto
    each device once at setup, outside the per-call path.
  - both outputs pack into one [128, 65] f32 tensor (cols 0-63 per-ref
    max partials, col 64 per-query sqrt-sum partials) -> one fetch RPC.
  - dispatch is 8 independent single-device jit calls issued from a
    thread pool (put -> exec -> fetch pipelined per device, overlapped
    across devices); the donated zero output buffers are prefetched
    and each core is submitted as soon as its host prep finishes.

kernel(pred, label) takes the full inputs, preps the compact augmented
layouts on host (cheap O(N*D) numpy), runs the 8 cores, and combines the
small per-core outputs.
"""

import os
import sys
from concurrent.futures import ThreadPoolExecutor

import ml_dtypes
import numpy as np

for _p in ("/opt/trn_rl_repo", "/root/.axon_site/_ro/trn_rl_repo"):
    if os.path.isdir(_p) and _p not in sys.path:
        sys.path.append(_p)

import concourse.bacc as bacc
import concourse.mybir as mybir
from concourse import tile

F32 = mybir.dt.float32
F16 = mybir.dt.float16
BF16 = mybir.dt.bfloat16
OP_MAX = mybir.AluOpType.max
AX_X = mybir.AxisListType.X
SQRT = mybir.ActivationFunctionType.Sqrt
COPY = mybir.ActivationFunctionType.Copy

B = 4
N = 8192
NCORES = 8
NEG16 = -60000.0

NQ = N // 2      # queries per core (pred half)
NR = N           # refs per core (all labels of the batch)
MMN = 512        # moving free dim per matmul (one PSUM bank)
K = 7            # fp16 augmented contraction dim
RT = NR // 128   # ref row-tiles (64)


def build_program(nq=NQ, nr=NR, mmn=MMN, dve_copy_every=5, scp_bufs=4,
                  gmm=4):
    """Emit + compile the per-core program.

    Input: aug [5, nq+nr] fp16. Columns 0:nq are the query block, rows
    {q.T(0-2), q2h(3), q2l(4)}; columns nq:nq+nr the ref block, rows
    {2r.T(0-2), -r2h(3), -r2l(4)}. The replicated 4-quadrant K=7 strips
    the PE needs (q side [q(3), 1, 1, q2h, q2l], r side
    [2r(3), -r2h, -r2l, -1, -1]) are built here by DMA + memset.
    Output: out [128, rt+2] bf16 = [RMS (bf16) | qsum (f32 bitcast)].
    """
    nchunk = gmm * mmn             # columns per consume group
    ngroup = nq // nchunk          # consume groups per ref row-tile
    rt = nr // 128                 # ref row-tiles
    psum_bufs = 8 // gmm           # PSUM slots (gmm banks each)
    assert nq % nchunk == 0 and nr % 128 == 0 and nq % 128 == 0

    S16 = BF16
    nc = bacc.Bacc("TRN2", target_bir_lowering=False, debug=False)
    aug_d = nc.dram_tensor("aug", [5, nq + nr], F16, kind="ExternalInput")
    id_d = nc.dram_tensor("ident", [128, 128], F32, kind="ExternalInput")
    # output is latency-sensitive (response-path bytes): RMS ships as
    # bf16, qsum stays exact f32 bitcast into the last two bf16 columns
    out_d = nc.dram_tensor("out", [128, rt + 2], BF16, kind="ExternalOutput")

    with tile.TileContext(nc) as tc:
        with (
            tc.tile_pool(name="const", bufs=1) as const,
            tc.tile_pool(name="rmp", bufs=2) as rmp,
            tc.tile_pool(name="scp", bufs=scp_bufs) as scp,
            tc.tile_pool(name="tail", bufs=1) as tail,
        ):
            QS = const.tile([128, nq // 4], F16)
            RS = const.tile([128, nr], F16)
            IDENT = const.tile([128, 128], F32)
            nc.sync.dma_start(IDENT[:], id_d.ap())

            # Strip assembly. qs[32s+k, j2*mmn+m] = qaug[k, (4j2+s)*mmn+m]
            # with qaug rows [q(3), 1, 1, q2h, q2l]; rs[32s+k, :] =
            # raug[k, :] with raug rows [2r(3), -r2h, -r2l, -1, -1].
            # Engines can't address partition slices off the 32-partition
            # grid, so pre-fill the constant rows by memsetting the whole
            # tiles, then DMA the data rows over (DMA has no partition
            # alignment constraint).
            nc.vector.memset(QS[:], 1.0)
            nc.vector.memset(RS[:], -1.0)
            qb = aug_d.ap()[0:5, 0:nq].rearrange(
                "r (g four m) -> r g four m", four=4, m=mmn)
            rb = aug_d.ap()[0:5, nq:nq + nr]
            for s in range(4):
                qdst = QS[32 * s:32 * s + 8, :].rearrange(
                    "p (g m) -> p g m", m=mmn)
                nc.sync.dma_start(qdst[0:3], qb[0:3, :, s, :])
                nc.sync.dma_start(qdst[5:7], qb[3:5, :, s, :])
                nc.sync.dma_start(RS[32 * s + 0:32 * s + 3, :], rb[0:3, :])
                nc.sync.dma_start(RS[32 * s + 3:32 * s + 5, :], rb[3:5, :])

            CM = const.tile([128, nq], S16)
            RMS = const.tile([128, rt], BF16)

            with tc.tile_pool(name="psum", bufs=psum_bufs, space="PSUM") as psum:
                nc.vector.memset(CM[:], NEG16)
                for r in range(rt):
                    # one contiguous copy target for the whole row tile
                    sc = scp.tile([128, nq], S16, tag="sc")
                    dve_rt = (ngroup > 1 and dve_copy_every
                              and r % dve_copy_every == 0)
                    rg0 = None
                    for j2 in range(ngroup):
                        ps = psum.tile([128, nchunk], F32)
                        for i in range(gmm):
                            nc.tensor.matmul(
                                ps[:, i * mmn:(i + 1) * mmn],
                                RS[32 * i:32 * i + K, r * 128:(r + 1) * 128],
                                QS[32 * i:32 * i + K, j2 * mmn:(j2 + 1) * mmn],
                                start=True,
                                stop=True,
                                tile_position=(32 * i, 0),
                            )
                        sc_sl = sc[:, j2 * nchunk:(j2 + 1) * nchunk]
                        if dve_rt and j2 == 0:
                            # DVE reads this PSUM group: fused copy+reduce
                            rg0 = rmp.tile([128, 1], F32, tag="rg0")
                            nc.vector.tensor_scalar(
                                sc_sl, ps[:], -3.0e38, None, OP_MAX, OP_MAX,
                                accum_out=rg0[:])
                        else:
                            nc.scalar.activation(sc_sl, ps[:], COPY)
                    # per-ref reduce over the whole row tile
                    if dve_rt:
                        rg1 = rmp.tile([128, 1], F32, tag="rg1")
                        nc.vector.reduce_max(rg1[:], sc[:, nchunk:], axis=AX_X)
                        nc.vector.tensor_max(RMS[:, r:r + 1], rg0[:], rg1[:])
                    else:
                        nc.vector.reduce_max(RMS[:, r:r + 1], sc[:], axis=AX_X)
                    # per-query fold into CM (full row-tile width)
                    nc.vector.tensor_max(CM[:], CM[:], sc[:])

                nc.sync.dma_start(out_d.ap()[:, 0:rt], RMS[:])

                # per-query direction: max over the 128 partitions of CM.
                # Clamp+convert CM16 -> fp32, PE-transpose each 128x128
                # block into PSUM, reduce_max over the candidate axis,
                # then sqrt of the negated minima and sum.
                CM32 = tail.tile([128, nq], F32)
                nc.vector.tensor_scalar_min(CM32[:], CM[:], 0.0)
                nblk = nq // 128
                nbp = nchunk // 128    # transpose blocks per pass
                q2 = tail.tile([128, nblk], F32)
                for h2 in range(nq // nchunk):
                    pst = psum.tile([128, nchunk], F32, tag="ps")
                    for bb in range(nbp):
                        blk = h2 * nbp + bb
                        nc.tensor.transpose(
                            pst[:, bb * 128:(bb + 1) * 128],
                            CM32[:, blk * 128:(blk + 1) * 128],
                            IDENT[:],
                        )
                    nc.vector.tensor_reduce(
                        q2[:, h2 * nbp:(h2 + 1) * nbp],
                        pst[:].rearrange("p (b c) -> p b c", c=128),
                        axis=AX_X, op=OP_MAX,
                    )
                # q2 holds v = max(-d^2) clamped <= 0; sqrt(-v) = distance.
                sq = tail.tile([128, nblk], F32)
                nc.scalar.activation(sq[:], q2[:], SQRT, bias=0.0, scale=-1.0)
                qsum = tail.tile([128, 1], F32)
                nc.vector.reduce_sum(qsum[:], sq[:], axis=AX_X)
                nc.sync.dma_start(out_d.ap()[:, rt:rt + 2],
                                  qsum[:].bitcast(BF16))

    nc.compile()
    return nc


def _split2_16(x):
    """fp32 -> (hi, lo) fp32 arrays exactly representable in fp16."""
    hi = x.astype(np.float16).astype(np.float32)
    lo = (x - hi).astype(np.float16).astype(np.float32)
    return hi, lo


def iter_augs(pred, label):
    """Yield (core, aug [5, NQ+NR] fp16) in core order."""
    pred = np.asarray(pred, np.float32)
    label = np.asarray(label, np.float32)
    for b in range(B):
        r16 = np.ascontiguousarray(label[b]).astype(np.float16)
        r32 = r16.astype(np.float32)
        r2h, r2l = _split2_16((r32 * r32).sum(1, dtype=np.float32))
        rblock = np.empty((5, NR), np.float16)
        rblock[0:3] = 2.0 * r16.T
        rblock[3] = -r2h
        rblock[4] = -r2l
        for h in range(2):
            q16 = np.ascontiguousarray(
                pred[b, h * NQ:(h + 1) * NQ]).astype(np.float16)
            q32 = q16.astype(np.float32)
            q2h, q2l = _split2_16((q32 * q32).sum(1, dtype=np.float32))
            aug = np.empty((5, NQ + NR), np.float16)
            aug[0:3, :NQ] = q16.T
            aug[3, :NQ] = q2h
            aug[4, :NQ] = q2l
            aug[:, NQ:] = rblock
            yield 2 * b + h, aug


def postprocess(outs):
    """outs: list of 8 [128, RT+2] bf16 arrays ([RMS bf16 | qsum f32])."""
    sq_sum = 0.0
    rms = []
    for o in outs:
        qsum = np.ascontiguousarray(o[:, RT:RT + 2]).view(np.float32)
        sq_sum += float(qsum.sum(dtype=np.float64))
        rms.append(o[:, :RT].astype(np.float32))
    ref_sum = 0.0
    for b in range(B):
        m = np.maximum(rms[2 * b], rms[2 * b + 1])
        ref_sum += float(np.sqrt(np.maximum(-m, 0.0)).sum(dtype=np.float64))
    return np.float32((sq_sum + ref_sum) / (B * N))


_CTX = None


def _setup():
    global _CTX
    if _CTX is not None:
        return _CTX
    import jax
    from concourse.bass2jax import (_bass_exec_p, install_neuronx_cc_hook,
                                    partition_id_tensor)

    nc = build_program()
    install_neuronx_cc_hook()

    partition_name = (nc.partition_id_tensor.name
                      if nc.partition_id_tensor else None)
    in_names, out_names, out_avals = [], [], []
    for alloc in nc.m.functions[0].allocations:
        if not isinstance(alloc, mybir.MemoryLocationSet):
            continue
        name = alloc.memorylocations[0].name
        if alloc.kind == "ExternalInput":
            if name != partition_name:
                in_names.append(name)
        elif alloc.kind == "ExternalOutput":
            shape = tuple(alloc.tensor_shape)
            dtype = mybir.dt.np(alloc.dtype)
            out_names.append(name)
            out_avals.append(jax.core.ShapedArray(shape, dtype))
    assert in_names == ["aug", "ident"] and out_names == ["out"], (
        in_names, out_names)
    all_in_names = in_names + out_names + (
        [partition_name] if partition_name else [])
    n_params = len(in_names)
    donate = tuple(range(n_params, n_params + len(out_names)))

    def _body(*args):
        operands = list(args)
        if partition_name is not None:
            operands.append(partition_id_tensor())
        outs = _bass_exec_p.bind(
            *operands, out_avals=tuple(out_avals),
            in_names=tuple(all_in_names), out_names=tuple(out_names),
            lowering_input_output_aliases=(), sim_require_finite=True,
            sim_require_nnan=True, nc=nc)
        return tuple(outs)

    jfn = jax.jit(_body, donate_argnums=donate, keep_unused=True)
    devices = jax.devices()[:NCORES]
    ident = np.eye(128, dtype=np.float32)
    ident_devs = [jax.device_put(ident, d) for d in devices]
    # warm/compile each device executable once (NEFF compile is cached)
    dummy = np.zeros((5, NQ + NR), np.float16)
    for c in range(NCORES):
        ad = jax.device_put(dummy, devices[c])
        (o,) = jfn(ad, ident_devs[c],
                   np.zeros((128, RT + 2), ml_dtypes.bfloat16))
        np.asarray(o)
    pool = ThreadPoolExecutor(NCORES)
    _CTX = {
        "jax": jax, "jfn": jfn, "devices": devices,
        "ident_devs": ident_devs, "pool": pool,
        "zeros": np.zeros((128, RT + 2), ml_dtypes.bfloat16),
    }
    return _CTX


def kernel(pred, label):
    ctx = _setup()
    jax, jfn = ctx["jax"], ctx["jfn"]
    devices, ident_devs = ctx["devices"], ctx["ident_devs"]
    pool, zeros = ctx["pool"], ctx["zeros"]

    # prefetch the donated zero output buffers (input-independent) so
    # their transfer streams while host prep runs
    zfuts = [pool.submit(jax.device_put, zeros, devices[c])
             for c in range(NCORES)]

    def run_core(c, aug):
        ad = jax.device_put(aug, devices[c])
        (o,) = jfn(ad, ident_devs[c], zfuts[c].result())
        return np.asarray(o)

    # submit each core as soon as its host prep is done
    futs = [None] * NCORES
    for c, aug in iter_augs(pred, label):
        futs[c] = pool.submit(run_core, c, aug)
    outs = [f.result() for f in futs]
    return postprocess(outs)
